# revision 42
# baseline (speedup 1.0000x reference)
"""Two-layer GAT (PyG GATConv, heads=3, concat=False/mean) on 8 trn2 NeuronCores.

Strategy (1D dest-partitioning):
  - dests sharded 6250/core; each core owns all edges INTO its dests.
  - dense projection H = X @ Wfold computed replicated per core into a
    per-core DRAM table (own dests first); rows are 256 bf16 (512B, the
    dma_gather minimum) but only cols 0:200 are written/used:
    [h(192) | ad(3) | as(3) | pad(2)].
  - per-edge source rows fetched with dma_gather; each (block, half) is
    split into two calls on separate SWDGE queues so four queues drain
    one block in parallel (int16 indices; the table is split into two
    <=25001-row half tensors, which also lets A-half gathers overlap the
    dense phase's B-half writes). Per-core shortfall vs the max-core
    count is padded with a zero dummy row (the ucode requires
    num_idxs_reg == #valid indices on every core).
  - self-loops are NOT gathered: each block's own dest rows are loaded
    contiguously (they also provide the dest-attention columns) and
    accumulated via an identity matmul.
  - one-hot S (edge->dest) and S_T (dest->edge) matrices are stored as
    fp8e4m3 in partition-major DRAM layout, HWDGE-loaded, and used
    directly as matmul weights against bf16 data (exact).
  - softmax: e = as + ad (DVE), p = Exp(Prelu_0.2(e)) (scalar engine),
    hp = h*p (DVE broadcast), numerator+denominator accumulated in PSUM
    by S matmuls; epilogue scales per head on the scalar engine with
    per-partition 1/(3*den) and sums heads on DVE.
  - two launches of ONE compiled program (layer 1, then layer 2 with the
    bias+relu'd output relayed through the host, zero-padded to 128).
"""
import sys

if '/opt/trn_rl_repo' not in sys.path:
    sys.path.insert(0, '/opt/trn_rl_repo')

import os
import types

import numpy as np
import ml_dtypes

import concourse.bass as bass
import concourse.bacc as bacc
import concourse.tile as tile
from concourse import mybir
from concourse.bass_utils import run_bass_kernel_spmd

timed_ns = None


def _try_install_profile_hook():
    """Optional: restore NTFF profiling (agent image lacks antenv.axon_hooks).
    Only used when GAT_PROFILE=1; failures are non-fatal."""
    try:
        if 'antenv.axon_hooks' in sys.modules:
            return True
        if '/root/.axon_site' not in sys.path:
            sys.path.insert(0, '/root/.axon_site')
        from trn_agent_boot.trn_boot import _ntff_profile_via_ctypes
        hook = _ntff_profile_via_ctypes('/opt/axon/libaxon_pjrt.so')
        mod = types.ModuleType('antenv.axon_hooks')
        mod.get_axon_ntff_profile_hook = lambda: hook
        mod.set_axon_ntff_profile_hook = lambda h: None
        import antenv
        sys.modules['antenv.axon_hooks'] = mod
        antenv.axon_hooks = mod
        from concourse import bass_utils
        bass_utils.upload_artifacts = lambda tmpdir: tmpdir
        return True
    except Exception:
        return False

BF16 = ml_dtypes.bfloat16
FP8 = ml_dtypes.float8_e4m3

N = 50000
IN_F = 128
HID = 64
HEADS = 3
NEG = 0.2
W = 8                 # cores
NLOC = N // W         # 6250 dests per core
P = 128
NBLK = (NLOC + P - 1) // P          # 49 dest blocks per core
ROW = 256                            # table row elems (bf16, 512B) - gather granularity
WCOL = 200                           # written/used columns per row
HALF = 25001                         # rows per table half
HALFP = 25088                        # padded half rows (196 tiles of 128)
NPAD = 2 * HALFP                     # staged xT columns (A half then B half)
FP8_ONE = np.uint8(0x38)             # fp8e4m3 encoding of 1.0

_printed = {}
NOACT = os.environ.get('GAT_NOACT') == '1'        # bisect: avoid new scalar-engine ops
PADGATHER = os.environ.get('GAT_PADGATHER') == '1'  # bisect: full num_idxs, dummy-row pad
NOFP8 = os.environ.get('GAT_NOFP8') == '1'          # bisect: S/ST as bf16
FULLROW = os.environ.get('GAT_FULLROW') == '1'      # bisect: write all 256 table cols
SIMQ0 = os.environ.get('GAT_SIMQ0') == '1'          # CoreSim: single queue (sim enforces
                                                    # a sem/queue lock that HW doesn't)
DUMA = NLOC   # zero row usable as A-half dummy gather target
DUMB = 0      # global row HALF (B-local 0) is likewise reserved/zero


def _round16(n):
    return (int(n) + 15) // 16 * 16


def _build_structure(edge_index):
    """Host-side: per-core edge chunking, index & one-hot tensors.

    Self-loops are excluded (handled by an on-chip identity chunk).
    Per (block, half) there is ONE gather call; per-core shortfall vs the
    max-core edge count is padded with -1 indices (skipped by the ucode).
    """
    src = np.asarray(edge_index[0]).astype(np.int64)
    dst = np.asarray(edge_index[1]).astype(np.int64)

    # per-core node->row map: own dests first, then the rest split across
    # the two halves (rows NLOC and HALF stay zero, unused).
    rowmap = np.empty((W, N), np.int64)
    for c in range(W):
        own = np.arange(c * NLOC, (c + 1) * NLOC)
        others = np.concatenate([np.arange(0, c * NLOC), np.arange((c + 1) * NLOC, N)])
        rowmap[c, own] = np.arange(NLOC)
        nA_rest = HALF - NLOC - 1
        rowmap[c, others[:nA_rest]] = NLOC + 1 + np.arange(nA_rest)
        rowmap[c, others[nA_rest:]] = HALF + 1 + np.arange(len(others) - nA_rest)

    core_of = dst // NLOC
    # per (core, block): A-half and B-half (row, dest-local) lists
    blk_edges = [[None] * NBLK for _ in range(W)]
    cntA = np.zeros((W, NBLK), np.int64)
    cntB = np.zeros((W, NBLK), np.int64)
    for c in range(W):
        sel = core_of == c
        es = src[sel]
        ed = dst[sel] - c * NLOC
        erow = rowmap[c, es]
        blk = ed // P
        for b in range(NBLK):
            m = blk == b
            er, dl = erow[m], ed[m] - b * P
            isA = er < HALF
            eA_r, eA_d = er[isA], dl[isA]
            eB_r, eB_d = er[~isA] - HALF, dl[~isA]
            oA = np.argsort(eA_r, kind='stable')
            oB = np.argsort(eB_r, kind='stable')
            blk_edges[c][b] = (eA_r[oA], eA_d[oA], eB_r[oB], eB_d[oB])
            cntA[c, b] = len(eA_r)
            cntB[c, b] = len(eB_r)

    # uniform (max-over-cores) gather sizes per (block, half)
    nA_u = np.maximum(16, np.vectorize(_round16)(cntA.max(axis=0)))
    nB_u = np.maximum(16, np.vectorize(_round16)(cntB.max(axis=0)))
    kA_u = (nA_u + P - 1) // P
    kB_u = (nB_u + P - 1) // P
    if PADGATHER:
        nA_u = kA_u * P
        nB_u = kB_u * P
    chunk_off = np.concatenate([[0], np.cumsum(kA_u + kB_u)])[:NBLK]
    C_total = int((kA_u + kB_u).sum())

    idx_cols = int(8 * (kA_u.sum() + kB_u.sum()))  # 8 cols per chunk
    # split each (block, half) into two gather calls on separate queues so
    # four SWDGE queues drain one block's edges in parallel.
    # call: (block, half, chunk_off_in_block, nch, nidx, queue, icol)
    calls = []
    icol = 0
    for b in range(NBLK):
        kA, kB, nA, nB = int(kA_u[b]), int(kB_u[b]), int(nA_u[b]), int(nB_u[b])
        for half, k_h, n_h, o_h, qbase in ((0, kA, nA, 0, 0), (1, kB, nB, kA, 2)):
            k1 = (k_h + 1) // 2
            k2 = k_h - k1
            if k2 == 0:
                calls.append((b, half, o_h, k_h, n_h, qbase, icol))
                icol += k_h * 8
            else:
                n1 = k1 * P
                calls.append((b, half, o_h, k1, n1, qbase, icol))
                icol += k1 * 8
                calls.append((b, half, o_h + k1, k2, n_h - n1, qbase + 1, icol))
                icol += k2 * 8
    out = {
        'kA': kA_u, 'kB': kB_u, 'nA': nA_u, 'nB': nB_u, 'calls': calls,
        'chunk_off': chunk_off, 'C_total': C_total, 'rowmap': rowmap,
        'idx16': np.full((W, P, idx_cols), -1, np.int16),
        'S': np.zeros((W, P, C_total * P), np.uint8),
        'ST': np.zeros((W, P, C_total * P), np.uint8),
    }
    for c in range(W):
        S = out['S'][c]
        ST = out['ST'][c]
        half_flat = {}   # (b, half) -> full-half slot->row array (dummy-padded)
        for b in range(NBLK):
            co = int(chunk_off[b])
            eA_r, eA_d, eB_r, eB_d = blk_edges[c][b]
            for (rows, dls, k0, nch, half) in (
                (eA_r, eA_d, co, int(kA_u[b]), 0),
                (eB_r, eB_d, co + int(kA_u[b]), int(kB_u[b]), 1),
            ):
                ne = len(rows)
                slot = np.arange(ne)
                ch = k0 + slot // P
                ee = slot % P
                S[ee, ch * P + dls] = FP8_ONE
                ST[dls, ch * P + ee] = FP8_ONE
                # Pad with the zero dummy row up to the uniform count: the
                # gather ucode requires num_idxs_reg == #(valid idxs) on
                # EVERY core, and the program (hence the immediate) is
                # shared, so the valid count must be core-uniform.
                dummy = DUMA if half == 0 else DUMB
                flat = np.full(nch * P, -1, np.int16)
                n_u = int(nA_u[b] if half == 0 else nB_u[b])
                flat[:n_u] = dummy
                flat[:ne] = rows.astype(np.int16)
                half_flat[(b, half)] = flat
        for (b, half, o_h, nch, nidx, q, icol) in calls:
            o0 = o_h - (0 if half == 0 else int(kA_u[b]))  # chunk offset in half
            flat = half_flat[(b, half)][o0 * P:(o0 + nch) * P].copy()
            wrapped = np.full((16, nch * 8), -1, np.int16)
            i = np.arange(nch * P)
            wrapped[i % 16, i // 16] = flat
            out['idx16'][c, :, icol:icol + nch * 8] = np.tile(wrapped, (8, 1))
    return out


def _fold_w(Wm, a_src, a_dst):
    in_f = Wm.shape[0]
    Wf = np.zeros((P, ROW if FULLROW else WCOL), np.float32)
    Wf[:in_f, 0:192] = Wm
    for h in range(HEADS):
        Wf[:in_f, 192 + h] = Wm[:, h * HID:(h + 1) * HID] @ a_dst[h]
        Wf[:in_f, 195 + h] = Wm[:, h * HID:(h + 1) * HID] @ a_src[h]
    return Wf.astype(BF16)


def _build_nc(st):
    """Build the (single) SPMD program."""
    kA, kB, nA, nB, chunk_off, C_total = (
        st['kA'], st['kB'], st['nA'], st['nB'], st['chunk_off'], st['C_total'])
    idx_cols = st['idx16'].shape[2]

    nc = bacc.Bacc("TRN2", target_bir_lowering=False, debug=False,
                   num_swdge_queues=4)
    xT_in = nc.declare_dram_parameter("xT", [P, NPAD], mybir.dt.bfloat16, isOutput=False)
    wf_in = nc.declare_dram_parameter("wf", [P, ROW if FULLROW else WCOL], mybir.dt.bfloat16, isOutput=False)
    SDT = mybir.dt.bfloat16 if NOFP8 else mybir.dt.float8e4
    s_in = nc.declare_dram_parameter("s_f8", [P, C_total * P], SDT, isOutput=False)
    st_in = nc.declare_dram_parameter("st_f8", [P, C_total * P], SDT, isOutput=False)
    idx_in = nc.declare_dram_parameter("idx16", [P, idx_cols], mybir.dt.int16, isOutput=False)
    eye_in = nc.declare_dram_parameter("eye", [P, P], mybir.dt.bfloat16, isOutput=False)
    out_raw = nc.declare_dram_parameter("out_raw", [NLOC, HID], mybir.dt.float32, isOutput=True)

    table_a = nc.dram_tensor("tableA", [HALFP, ROW], mybir.dt.bfloat16)
    table_b = nc.dram_tensor("tableB", [HALFP, ROW], mybir.dt.bfloat16)

    DT = mybir.dt.bfloat16
    F8 = mybir.dt.float8e4
    F32 = mybir.dt.float32
    n_htile = HALFP // P              # 196 node tiles per half
    DGRP = 8                          # dense tiles per DMA group
    KBMAX = int((kA + kB).max())      # uniform hg tile size (stale tails stay finite)
    WC = ROW if FULLROW else WCOL

    with tile.TileContext(nc) as tc:
        with (
            tc.tile_pool(name="const", bufs=1) as cpool,
            tc.tile_pool(name="dense", bufs=3) as dpool,
            tc.tile_pool(name="dpsum", bufs=4, space="PSUM") as dpsum,
            tc.tile_pool(name="gath", bufs=3) as gpool,
            tc.tile_pool(name="smat", bufs=2) as spool,
            tc.tile_pool(name="selfp", bufs=2) as fpool,
            tc.tile_pool(name="blk", bufs=3) as bpool,
            tc.tile_pool(name="apsum", bufs=2, space="PSUM") as apsum,
            tc.tile_pool(name="adpsum", bufs=2, space="PSUM") as adpsum,
        ):
            wf_t = cpool.tile([P, ROW if FULLROW else WCOL], DT)
            nc.sync.dma_start(out=wf_t[:], in_=wf_in[:])
            idx_t = cpool.tile([P, idx_cols], mybir.dt.int16)
            nc.sync.dma_start(out=idx_t[:], in_=idx_in[:])
            eye_t = cpool.tile([P, P], DT)
            nc.sync.dma_start(out=eye_t[:], in_=eye_in[:])

            # ---- dense phase: table[:, 0:200] = xT.T @ wf, A half then B ----
            for hf, table in ((0, table_a), (1, table_b)):
                for g0 in range(0, n_htile, DGRP):
                    g1 = min(g0 + DGRP, n_htile)
                    ng = g1 - g0
                    x0 = hf * HALFP + g0 * P
                    xg = dpool.tile([P, DGRP * P], DT, tag="xg")
                    nc.sync.dma_start(out=xg[:, :ng * P], in_=xT_in[:, x0:x0 + ng * P])
                    hg_stage = dpool.tile([P, DGRP * WC], DT, tag="hstage")
                    for t in range(g0, g1):
                        ps = dpsum.tile([P, WC], F32)
                        nc.tensor.matmul(out=ps[:], lhsT=xg[:, (t - g0) * P:(t - g0 + 1) * P],
                                         rhs=wf_t[:], start=True, stop=True)
                        dst_sl = hg_stage[:, (t - g0) * WC:(t - g0 + 1) * WC]
                        if NOACT or t % 2 == 0:
                            nc.vector.tensor_copy(out=dst_sl, in_=ps[:])
                        else:
                            nc.scalar.copy(out=dst_sl, in_=ps[:])
                    nc.sync.dma_start(
                        out=table[g0 * P:g1 * P, 0:WC].rearrange("(t p) r -> p t r", p=P),
                        in_=hg_stage[:, :ng * WC].rearrange("p (t r) -> p t r", r=WC),
                    )

            # ---- aggregation phase ----
            calls_by_block = {b: [] for b in range(NBLK)}
            for call in st['calls']:
                calls_by_block[call[0]].append(call)

            for b in range(NBLK):
                kb = int(kA[b] + kB[b])
                co = int(chunk_off[b])
                ndest = min(P, NLOC - b * P)

                # self rows: h + dest attention for this block
                self_t = fpool.tile([P, WCOL], DT, tag="self")
                if ndest < P:
                    nc.vector.memset(self_t[:], 0.0)
                nc.sync.dma_start(out=self_t[:ndest, :], in_=table_a[b * P:b * P + ndest, 0:WCOL])

                hg = gpool.tile([P, KBMAX * ROW], DT, tag="hg")
                hg3 = hg[:].rearrange("p (k r) -> p k r", r=ROW)
                for (_, half, o, nch, nidx, q, icol) in calls_by_block[b]:
                    # slots [nidx, nch*128) of this call are never written by
                    # the gather; pre-zero the last chunk (pool slots are
                    # ring-allocated, so leftover bytes are arbitrary). The
                    # gather overwrites the valid slots afterwards.
                    if nidx < nch * P:
                        nc.scalar.memzero(hg3[:, o + nch - 1:o + nch, :])
                    table_h = table_a if half == 0 else table_b
                    nc.gpsimd.dma_gather(
                        out_ap=hg3[:, o:o + nch, :],
                        in_ap=table_h[0:HALF, :],
                        idxs_ap=idx_t[:, icol:icol + ((nidx + 15) // 16)],
                        num_idxs=nidx,
                        num_idxs_reg=nidx,
                        elem_size=ROW,
                        queue_num=0 if SIMQ0 else (q + 2 * (b % 2)) % 4,
                        single_packet=True,
                    )

                s_t = spool.tile([P, kb * P], SDT, tag="s")
                nc.sync.dma_start(out=s_t[:], in_=s_in[:, co * P:(co + kb) * P])
                st_t = spool.tile([P, kb * P], SDT, tag="st")
                nc.sync.dma_start(out=st_t[:], in_=st_in[:, co * P:(co + kb) * P])

                # dest-attention broadcast to edges: ad_ps[:, j*3:(j+1)*3] = ST_j.T @ ad
                ad_ps = adpsum.tile([P, 64], F32)
                for j in range(kb):
                    nc.tensor.matmul(out=ad_ps[:, j * 3:(j + 1) * 3],
                                     lhsT=st_t[:, j * P:(j + 1) * P],
                                     rhs=self_t[:, 192:195], start=True, stop=True)

                # self-loop attention first (feeds the final eye matmul)
                es_t = bpool.tile([P, 3], F32, tag="es")
                nc.vector.tensor_tensor(out=es_t[:], in0=self_t[:, 195:198],
                                        in1=self_t[:, 192:195], op=mybir.AluOpType.add)
                ls_t = bpool.tile([P, 3], F32, tag="ls")
                if NOACT:
                    nc.vector.tensor_scalar_mul(ls_t[:], es_t[:], NEG)
                    nc.vector.tensor_tensor(out=ls_t[:], in0=ls_t[:], in1=es_t[:],
                                            op=mybir.AluOpType.max)
                else:
                    nc.scalar.activation(ls_t[:], es_t[:], mybir.ActivationFunctionType.Prelu,
                                         alpha=NEG)
                ps3 = bpool.tile([P, 3], F32, tag="ps3")
                nc.scalar.activation(ps3[:], ls_t[:], mybir.ActivationFunctionType.Exp)
                # p_self into self_t[.,192:195] (after ad/e_s reads) for the denominator
                if NOACT:
                    nc.vector.tensor_copy(out=self_t[:, 192:195], in_=ps3[:])
                else:
                    nc.scalar.copy(out=self_t[:, 192:195], in_=ps3[:])
                s4 = self_t[:, 0:3 * HID].rearrange("p (h c) -> p h c", c=HID)
                nc.vector.tensor_tensor(
                    out=s4[:, 0:3, :],
                    in0=s4[:, 0:3, :],
                    in1=ps3[:].unsqueeze(2).broadcast_to([P, 3, HID]),
                    op=mybir.AluOpType.mult,
                )

                # per-half attention chain: A's e/exp/hp/accumulation overlap
                # B's gather drain.
                acc = apsum.tile([P, 208], F32)
                hg4 = hg[:].rearrange("p (k h c) -> p k h c", h=4, c=HID)
                for (o_h, k_h) in ((0, int(kA[b])), (int(kA[b]), int(kB[b]))):
                    e_t = bpool.tile([P, k_h * 3], F32, tag=f"e{o_h == 0}")
                    nc.vector.tensor_tensor(out=e_t[:],
                                            in0=hg3[:, o_h:o_h + k_h, 195:198],
                                            in1=ad_ps[:, o_h * 3:(o_h + k_h) * 3],
                                            op=mybir.AluOpType.add)
                    lr_t = bpool.tile([P, k_h * 3], F32, tag=f"lr{o_h == 0}")
                    if NOACT:
                        nc.vector.tensor_scalar_mul(lr_t[:], e_t[:], NEG)
                        nc.vector.tensor_tensor(out=lr_t[:], in0=lr_t[:], in1=e_t[:],
                                                op=mybir.AluOpType.max)
                    else:
                        nc.scalar.activation(lr_t[:], e_t[:],
                                             mybir.ActivationFunctionType.Prelu, alpha=NEG)
                    nc.scalar.activation(hg3[:, o_h:o_h + k_h, 192:195],
                                         lr_t[:].rearrange("p (k t) -> p k t", t=3),
                                         mybir.ActivationFunctionType.Exp)
                    p4 = hg3[:, o_h:o_h + k_h, 192:195].unsqueeze(3)
                    nc.vector.tensor_tensor(
                        out=hg4[:, o_h:o_h + k_h, 0:3, :],
                        in0=hg4[:, o_h:o_h + k_h, 0:3, :],
                        in1=p4.broadcast_to([P, k_h, 3, HID]),
                        op=mybir.AluOpType.mult,
                    )
                    for j in range(o_h, o_h + k_h):
                        nc.tensor.matmul(out=acc[:, 0:195],
                                         lhsT=s_t[:, j * P:(j + 1) * P],
                                         rhs=hg3[:, j, 0:195],
                                         start=(j == 0), stop=False)
                nc.tensor.matmul(out=acc[:, 0:195], lhsT=eye_t[:],
                                 rhs=self_t[:, 0:195], start=False, stop=True)

                # epilogue: out = sum_h acc_h * (1/(3*den_h))
                den3 = bpool.tile([P, 3], F32, tag="den")
                nc.vector.tensor_scalar_mul(den3[:], acc[:, 192:195], 3.0)
                rec = bpool.tile([P, 3], F32, tag="rec")
                nc.vector.reciprocal(out=rec[:], in_=den3[:])
                o_parts = bpool.tile([P, HEADS * HID], F32, tag="op")
                for h in range(HEADS):
                    if NOACT:
                        nc.vector.tensor_tensor(
                            out=o_parts[:, h * HID:(h + 1) * HID],
                            in0=acc[:, h * HID:(h + 1) * HID],
                            in1=rec[:, h:h + 1].broadcast_to([P, HID]),
                            op=mybir.AluOpType.mult)
                    else:
                        nc.scalar.mul(o_parts[:, h * HID:(h + 1) * HID],
                                      acc[:, h * HID:(h + 1) * HID], rec[:, h:h + 1])
                o_raw = bpool.tile([P, HID], F32, tag="oraw")
                nc.vector.tensor_tensor(out=o_raw[:], in0=o_parts[:, 0:HID],
                                        in1=o_parts[:, HID:2 * HID], op=mybir.AluOpType.add)
                nc.vector.tensor_tensor(out=o_raw[:], in0=o_raw[:],
                                        in1=o_parts[:, 2 * HID:3 * HID], op=mybir.AluOpType.add)
                nc.sync.dma_start(out=out_raw[b * P:b * P + ndest, :], in_=o_raw[:ndest, :])

    nc.compile()
    return nc


def kernel(**inputs):
    x = np.asarray(inputs['x'], np.float32)
    edge_index = np.asarray(inputs['edge_index'])
    st = _build_structure(edge_index)
    nc = _build_nc(st)

    rowmap = st['rowmap']
    eye = np.eye(P, dtype=BF16)

    def xT_for(core, feats):
        in_f = feats.shape[1]
        rm = rowmap[core]
        slot = np.where(rm < HALF, rm, rm - HALF + HALFP)
        xsh = np.zeros((NPAD, P), BF16)
        xsh[slot, :in_f] = feats.astype(BF16)
        return np.ascontiguousarray(xsh.T)

    def run_layer(feats, Wm, a_src, a_dst):
        wf = _fold_w(np.asarray(Wm, np.float32),
                     np.asarray(a_src, np.float32), np.asarray(a_dst, np.float32))
        in_maps = []
        for c in range(W):
            in_maps.append({
                'xT': xT_for(c, feats),
                'wf': wf,
                's_f8': ((st['S'][c] != 0).astype(BF16) if NOFP8
                         else st['S'][c].view(FP8)),
                'st_f8': ((st['ST'][c] != 0).astype(BF16) if NOFP8
                          else st['ST'][c].view(FP8)),
                'idx16': st['idx16'][c],
                'eye': eye,
            })
        trace = os.environ.get('GAT_PROFILE') == '1' and _try_install_profile_hook()
        res = run_bass_kernel_spmd(nc, in_maps, core_ids=list(range(W)), trace=trace)
        global timed_ns
        if trace and res.exec_time_ns:
            timed_ns = (timed_ns or 0) + res.exec_time_ns
        return np.concatenate([res.results[c]['out_raw'] for c in range(W)], axis=0)

    raw1 = run_layer(x, inputs['W1'], inputs['att_src1'], inputs['att_dst1'])
    h1 = np.maximum(raw1 + np.asarray(inputs['bias1'], np.float32)[None, :], 0.0)
    out = run_layer(h1, inputs['W2'], inputs['att_src2'], inputs['att_dst2'])
    out = out + np.asarray(inputs['bias2'], np.float32)[None, :]
    return out.astype(np.float32)
